# revision 1
# baseline (speedup 1.0000x reference)
"""Trainium2 Bass kernel for Co-occurrence Infused Multi-Label Attention.

Shards the n_classes (code) axis across 8 NeuronCores; [token, class]
orientation so the softmax-weighted token contraction runs on the PE.

Per core (c = class shard of 1152, z = head, b = chunk, t = token):
  QgT [tf, c]  = tanh(trans_wT @ QT + b_tr)     (PE + ACT, tf on partitions)
  qT  [zh, c]  = q_wT @ QgT + q_b               (PE + DVE bias-add)
  QwTplus      = [per-z W_wT @ QgT ; ones]      (PE + DVE copy), [65, z*512+c]
  WKT [zh, t]  = tanh(k_wT @ HT + k_b)          (PE + ACT, bias per partition)
  WVplus [t, z*65+h] = [tanh(HT.T @ v_wT + v_b) ; ones]  (v_b via rank-1 MM)
  per (c-chunk of 512, b, z):
    scoresT[t, c] = WKT_z.T @ qT_z              (4 tchunk MMs, K=64)
    expT          = ACT Exp (PSUM->SBUF bf16, [128, 2w] instrs)
    Y [65, c]     = sum_t WVplus_z.T @ expT     (4 MMs K=128; row 64 = denom)
    prod [65, c]  = Y * QwTplus_z               (DVE, the only big DVE op)
    RD[0:32, 0:w]   += selR_idx.T @ prod        (PE: row idx = numerator)
    RD[0:32, 512:+w] += selD_idx.T @ prod       (PE: row idx = denominator)
  normR = RD[:, 0:w] * recip(RD[:, 512:+w])     (DVE, tiny)
  out [4, c] = zsel.T @ normR                   (PE, sums over z)
"""

import numpy as np
import ml_dtypes

# Problem constants (hardcoded per harness contract)
C_FULL = 8929
D = 768          # d_model
TF = 512         # transform dim (= NH * DK)
NH = 8           # heads
DK = 64          # head dim
B = 4            # chunks
T = 512          # tokens per chunk
BT = B * T       # 2048
N_CORES = 8
CP = 9216        # padded classes (8 * 1152)
CS = CP // N_CORES   # 1152 classes per core
NDC = D // 128       # 6 d-model chunks
NFC = TF // 128      # 4 transform chunks
NTT = BT // 128      # 16 token tiles
NPAIR = B * NH       # 32 (b,z) pairs
C_CHUNKS = [(0, 512), (512, 512), (1024, 128)]  # (offset, width) per core
SELW = 32 * 32 * 2 + 4   # selector tensor width (R blocks, D blocks, zsel)

_BF = ml_dtypes.bfloat16

_CACHE = {}


def _make_sel():
    """Selector constants [65, 2052]: per-pair numerator col blocks
    (cols idx*32..), denominator blocks (cols 1024+idx*32..), and the
    z-sum selector (cols 2048..2052, idx = z*4+b)."""
    sel = np.zeros((65, SELW), np.float32)
    for idx in range(NPAIR):
        sel[0:64, idx * 32 + idx] = 1.0            # numerator: sum rows 0-63
        sel[64, 1024 + idx * 32 + idx] = 1.0       # denominator: row 64
    for r in range(NPAIR):
        sel[r, 2048 + (r % 4)] = 1.0               # z-sum: idx = z*4+b
    return sel.astype(_BF)


def _build(a_zero: bool, reps: int = 1, zpair: bool = False, dma_spread: bool = True):
    from contextlib import ExitStack
    import concourse.bass as bass
    import concourse.mybir as mybir
    import concourse.tile as tile
    from concourse import bacc

    bf = mybir.dt.bfloat16
    f32 = mybir.dt.float32
    AF = mybir.ActivationFunctionType
    ALU = mybir.AluOpType

    nc = bacc.Bacc()

    qt_d = nc.declare_dram_parameter("qt", [D, CS], bf, isOutput=False)
    ht_d = nc.declare_dram_parameter("ht", [D, BT], bf, isOutput=False)
    wtr_d = nc.declare_dram_parameter("wtr", [D, TF], bf, isOutput=False)
    wq_d = nc.declare_dram_parameter("wq", [TF, TF], bf, isOutput=False)
    wk_d = nc.declare_dram_parameter("wk", [D, TF], bf, isOutput=False)
    wv_d = nc.declare_dram_parameter("wv", [D, TF], bf, isOutput=False)
    ww_d = nc.declare_dram_parameter("ww", [TF, TF], bf, isOutput=False)
    btr_d = nc.declare_dram_parameter("btr", [TF], f32, isOutput=False)
    bq_d = nc.declare_dram_parameter("bq", [TF], f32, isOutput=False)
    bk_d = nc.declare_dram_parameter("bk", [TF], f32, isOutput=False)
    bvb_d = nc.declare_dram_parameter("bvb", [1, TF], bf, isOutput=False)
    sel_d = nc.declare_dram_parameter("sel", [65, SELW], bf, isOutput=False)
    ea_d = None
    if not a_zero:
        ea_d = nc.declare_dram_parameter("ea", [128, NTT], f32, isOutput=False)
    out_d = nc.declare_dram_parameter("out", [B, CS], f32, isOutput=True)

    with tile.TileContext(nc) as tc, ExitStack() as top:
        const = top.enter_context(tc.tile_pool(name="const", bufs=1))

        # --- load weights / H / biases / selectors ---
        w_tr = const.tile([128, NDC * TF], bf)
        w_k = const.tile([128, NDC * TF], bf)
        w_v = const.tile([128, NDC * TF], bf)
        _dmae = [nc.sync, nc.scalar] if dma_spread else [nc.sync]
        _dmai = [0]
        def _dma(out, in_):
            _dmae[_dmai[0] % len(_dmae)].dma_start(out, in_)
            _dmai[0] += 1
        for j in range(NDC):
            _dma(w_tr[:, j * TF:(j + 1) * TF], wtr_d[j * 128:(j + 1) * 128, :])
            _dma(w_k[:, j * TF:(j + 1) * TF], wk_d[j * 128:(j + 1) * 128, :])
            _dma(w_v[:, j * TF:(j + 1) * TF], wv_d[j * 128:(j + 1) * 128, :])
        w_q = const.tile([128, NFC * TF], bf)
        w_W = const.tile([128, NFC * TF], bf)
        for j in range(NFC):
            _dma(w_q[:, j * TF:(j + 1) * TF], wq_d[j * 128:(j + 1) * 128, :])
            _dma(w_W[:, j * TF:(j + 1) * TF], ww_d[j * 128:(j + 1) * 128, :])
        ht_sb = const.tile([128, NDC * BT], bf)
        for j in range(NDC):
            _dma(ht_sb[:, j * BT:(j + 1) * BT], ht_d[j * 128:(j + 1) * 128, :])
        b_tr = const.tile([128, NFC], f32)
        b_q = const.tile([128, NFC], f32)
        b_k = const.tile([128, NFC], f32)
        nc.sync.dma_start(b_tr[:], btr_d[:].rearrange("(c p) -> p c", p=128))
        nc.sync.dma_start(b_q[:], bq_d[:].rearrange("(c p) -> p c", p=128))
        nc.sync.dma_start(b_k[:], bk_d[:].rearrange("(c p) -> p c", p=128))
        bvb = const.tile([1, TF], bf)
        nc.sync.dma_start(bvb[:], bvb_d[:, :])
        sel = const.tile([65, SELW], bf)
        nc.sync.dma_start(sel[:], sel_d[:, :])
        ones1 = const.tile([1, 128], bf)
        nc.gpsimd.memset(ones1[:], 1.0)
        ea_sb = None
        if not a_zero:
            ea_sb = const.tile([128, NTT], f32)
            nc.sync.dma_start(ea_sb[:], ea_d[:, :])

        # --- WKT [zh, t] = tanh(k_wT @ HT + k_b) ---
        wkt = const.tile([128, NFC * BT], bf)
        # --- WVplus [t, z*65+h], one [128, 520] block per token tile ---
        wvp = const.tile([128, NTT * 520], bf)
        for jt in range(NTT):
            for z in range(NH):
                nc.gpsimd.memset(wvp[:, jt * 520 + z * 65 + 64: jt * 520 + z * 65 + 65], 1.0)

        with ExitStack() as main:
            qin = main.enter_context(tc.tile_pool(name="qin", bufs=2))
            qg = main.enter_context(tc.tile_pool(name="qg", bufs=2))
            if not zpair:
                chps = main.enter_context(tc.tile_pool(name="chps", bufs=1, space="PSUM"))
            scps = main.enter_context(tc.tile_pool(name="scps", bufs=2, space="PSUM"))
            yps = main.enter_context(tc.tile_pool(name="yps", bufs=2 if zpair else 1, space="PSUM"))
            def chain_ps():
                if zpair:
                    t = scps.tile([128, 1024], f32, tag="psc", name="chainps")
                    return t
                t = chps.tile([128, 512], f32, tag="chain", name="chainps")
                return t
            rdps = main.enter_context(tc.tile_pool(name="rdps", bufs=1, space="PSUM"))
            expp = main.enter_context(tc.tile_pool(name="expp", bufs=6))
            prodp = main.enter_context(tc.tile_pool(name="prodp", bufs=4))
            tailp = main.enter_context(tc.tile_pool(name="tailp", bufs=2))
            outp = main.enter_context(tc.tile_pool(name="outp", bufs=2))

            for rep in range(reps):
                # --- K/V transform (psum slots shared with scores pool) ---
                for jz in range(NFC):
                    for jp in range(BT // 1024):
                        ps = scps.tile([128, 1024], f32, tag="psc")
                        for half in range(2):
                            jt = jp * 2 + half
                            for jd in range(NDC):
                                nc.tensor.matmul(
                                    ps[:, half * 512: half * 512 + 512],
                                    w_k[:, jd * TF + jz * 128: jd * TF + (jz + 1) * 128],
                                    ht_sb[:, jd * BT + jt * 512: jd * BT + (jt + 1) * 512],
                                    start=(jd == 0), stop=(jd == NDC - 1))
                        nc.scalar.activation(
                            wkt[:, jz * BT + jp * 1024: jz * BT + (jp + 1) * 1024],
                            ps[:, 0:1024], AF.Tanh, bias=b_k[:, jz:jz + 1])
                for jt in range(NTT):
                    ps = scps.tile([128, 1024], f32, tag="psc")
                    for jd in range(NDC):
                        nc.tensor.matmul(
                            ps[:, 0:512],
                            ht_sb[:, jd * BT + jt * 128: jd * BT + (jt + 1) * 128],
                            w_v[:, jd * TF:(jd + 1) * TF],
                            start=(jd == 0), stop=False)
                    nc.tensor.matmul(ps[:, 0:512], ones1[0:1, :], bvb[0:1, :],
                                     start=False, stop=True)
                    wvp_z = wvp[:, jt * 520: (jt + 1) * 520].rearrange(
                        "p (z h) -> p z h", h=65)
                    nc.scalar.activation(
                        wvp_z[:, :, 0:64],
                        ps[:, 0:512].rearrange("p (z h) -> p z h", h=64),
                        AF.Tanh)

                for (c0, w) in C_CHUNKS:
                    qt_sb = qin.tile([128, NDC * 512], bf, tag="qt")
                    for jd in range(NDC):
                        nc.sync.dma_start(qt_sb[:, jd * 512: jd * 512 + w],
                                          qt_d[jd * 128:(jd + 1) * 128, c0:c0 + w])
                    # QgT [tf, c] = tanh(trans_wT @ QT + b_tr)
                    qgt = qg.tile([128, NFC * 512], bf, tag="qgt")
                    for jf in range(NFC):
                        ps = chain_ps()
                        for jd in range(NDC):
                            nc.tensor.matmul(
                                ps[:, :w],
                                w_tr[:, jd * TF + jf * 128: jd * TF + (jf + 1) * 128],
                                qt_sb[:, jd * 512: jd * 512 + w],
                                start=(jd == 0), stop=(jd == NDC - 1))
                        nc.scalar.activation(qgt[:, jf * 512: jf * 512 + w], ps[:, :w],
                                             AF.Tanh, bias=b_tr[:, jf:jf + 1])
                    # qT [zh, c] = q_wT @ QgT + q_b  (bias-add on DVE)
                    qtt = qg.tile([128, NFC * 512], bf, tag="qtt")
                    for jz in range(NFC):
                        ps = chain_ps()
                        for jf in range(NFC):
                            nc.tensor.matmul(
                                ps[:, :w],
                                w_q[:, jf * TF + jz * 128: jf * TF + (jz + 1) * 128],
                                qgt[:, jf * 512: jf * 512 + w],
                                start=(jf == 0), stop=(jf == NFC - 1))
                        nc.vector.tensor_scalar_add(qtt[:, jz * 512: jz * 512 + w],
                                                    ps[:, :w], b_q[:, jz:jz + 1])
                    # QwTplus [65, z*512+c]: rows 0-63 per-z W_wT@QgT, row 64 ones
                    qwtp = qg.tile([65, NH * 512], bf, tag="qwtp")
                    nc.gpsimd.memset(qwtp[64:65, :], 1.0)
                    for z in range(NH):
                        jz, hz = z // 2, (z % 2) * 64
                        ps = chain_ps()
                        for jf in range(NFC):
                            nc.tensor.matmul(
                                ps[0:64, :w],
                                w_W[:, jf * TF + jz * 128 + hz: jf * TF + jz * 128 + hz + 64],
                                qgt[:, jf * 512: jf * 512 + w],
                                start=(jf == 0), stop=(jf == NFC - 1))
                        nc.vector.tensor_copy(qwtp[0:64, z * 512: z * 512 + w],
                                              ps[0:64, :w])

                    # attention pairs
                    rd = rdps.tile([32, 1024], f32, tag="rd")
                    if zpair:
                        for step in range(NPAIR // 2):
                            bb = step // (NH // 2)
                            jz = step % (NH // 2)
                            ys = [None, None]
                            for half in range(2):
                                psc_a = scps.tile([128, 1024], f32, tag="psc")
                                psc_b = scps.tile([128, 1024], f32, tag="psc")
                                pscs = [psc_a, psc_b]
                                for slot in range(2):
                                    jt = half * 2 + slot
                                    for zi in range(2):
                                        hz = zi * 64
                                        nc.tensor.matmul(
                                            pscs[zi][:, slot * w: slot * w + w],
                                            wkt[hz:hz + 64,
                                                jz * BT + bb * 512 + jt * 128:
                                                jz * BT + bb * 512 + (jt + 1) * 128],
                                            qtt[hz:hz + 64, jz * 512: jz * 512 + w],
                                            start=True, stop=True)
                                for zi in range(2):
                                    z = jz * 2 + zi
                                    et = expp.tile([128, 1024], bf, tag="et")
                                    nc.scalar.activation(et[:, 0:2 * w],
                                                         pscs[zi][:, 0:2 * w], AF.Exp)
                                    if not a_zero:
                                        et2 = expp.tile([128, 1024], bf, tag="et2")
                                        for slot in range(2):
                                            jt = half * 2 + slot
                                            nc.vector.tensor_scalar_mul(
                                                et2[:, slot * w: slot * w + w],
                                                et[:, slot * w: slot * w + w],
                                                ea_sb[:, bb * 4 + jt: bb * 4 + jt + 1])
                                        et = et2
                                    if half == 0:
                                        yv = yps.tile([65, 512], f32, tag="y")
                                        ys[zi] = yv
                                    for slot in range(2):
                                        jt = half * 2 + slot
                                        gt = bb * 4 + jt
                                        nc.tensor.matmul(
                                            ys[zi][:, :w],
                                            wvp[:, gt * 520 + z * 65: gt * 520 + (z + 1) * 65],
                                            et[:, slot * w: slot * w + w],
                                            start=(jt == 0), stop=(jt == 3))
                            for zi in range(2):
                                z = jz * 2 + zi
                                idx = z * B + bb
                                prod = prodp.tile([65, 512], bf, tag="prod")
                                nc.vector.tensor_mul(prod[:, :w], ys[zi][:, :w],
                                                     qwtp[:, z * 512: z * 512 + w])
                                first = (step == 0 and zi == 0)
                                last = (step == NPAIR // 2 - 1 and zi == 1)
                                nc.tensor.matmul(rd[:, 0:w],
                                                 sel[:, idx * 32: (idx + 1) * 32],
                                                 prod[:, :w],
                                                 start=first, stop=last)
                                nc.tensor.matmul(rd[:, 512: 512 + w],
                                                 sel[:, 1024 + idx * 32: 1024 + (idx + 1) * 32],
                                                 prod[:, :w],
                                                 start=first, stop=last)
                    else:
                     for pair in range(NPAIR):
                        z = pair % NH
                        bb = pair // NH
                        jz, hz = z // 2, (z % 2) * 64
                        idx = z * B + bb
                        for half in range(2):
                            psc = scps.tile([128, 1024], f32, tag="psc")
                            for slot in range(2):
                                jt = half * 2 + slot
                                nc.tensor.matmul(
                                    psc[:, slot * w: slot * w + w],
                                    wkt[hz:hz + 64,
                                        jz * BT + bb * 512 + jt * 128:
                                        jz * BT + bb * 512 + (jt + 1) * 128],
                                    qtt[hz:hz + 64, jz * 512: jz * 512 + w],
                                    start=True, stop=True)
                            et = expp.tile([128, 1024], bf, tag="et")
                            nc.scalar.activation(et[:, 0:2 * w], psc[:, 0:2 * w], AF.Exp)
                            if not a_zero:
                                et2 = expp.tile([128, 1024], bf, tag="et2")
                                for slot in range(2):
                                    jt = half * 2 + slot
                                    nc.vector.tensor_scalar_mul(
                                        et2[:, slot * w: slot * w + w],
                                        et[:, slot * w: slot * w + w],
                                        ea_sb[:, bb * 4 + jt: bb * 4 + jt + 1])
                                et = et2
                            if half == 0:
                                y = yps.tile([65, 512], f32, tag="y")
                            for slot in range(2):
                                jt = half * 2 + slot
                                gt = bb * 4 + jt
                                nc.tensor.matmul(
                                    y[:, :w],
                                    wvp[:, gt * 520 + z * 65: gt * 520 + (z + 1) * 65],
                                    et[:, slot * w: slot * w + w],
                                    start=(jt == 0), stop=(jt == 3))
                        prod = prodp.tile([65, 512], bf, tag="prod")
                        nc.vector.tensor_mul(prod[:, :w], y[:, :w],
                                             qwtp[:, z * 512: z * 512 + w])
                        nc.tensor.matmul(rd[:, 0:w],
                                         sel[:, idx * 32: (idx + 1) * 32],
                                         prod[:, :w],
                                         start=(pair == 0), stop=(pair == NPAIR - 1))
                        nc.tensor.matmul(rd[:, 512: 512 + w],
                                         sel[:, 1024 + idx * 32: 1024 + (idx + 1) * 32],
                                         prod[:, :w],
                                         start=(pair == 0), stop=(pair == NPAIR - 1))

                    # tail: normalize and z-sum
                    rden = tailp.tile([32, 512], f32, tag="rden")
                    nc.vector.reciprocal(rden[:, :w], rd[:, 512: 512 + w])
                    normr = tailp.tile([32, 512], bf, tag="normr")
                    nc.vector.tensor_mul(normr[:, :w], rd[:, 0:w], rden[:, :w])
                    if zpair:
                        zs = rdps.tile([32, 1024], f32, tag="rd")
                    else:
                        zs = chps.tile([128, 512], f32, tag="chain")
                    nc.tensor.matmul(zs[0:4, :w], sel[0:32, 2048:2052], normr[:, :w],
                                     start=True, stop=True)
                    ot = outp.tile([4, 512], f32, tag="ot")
                    nc.vector.tensor_copy(ot[:, :w], zs[0:4, :w])
                    nc.sync.dma_start(out_d[:, c0:c0 + w], ot[:, :w])

    nc.compile()
    return nc


def _get_nc(a_zero: bool):
    key = ("nc", a_zero)
    if key not in _CACHE:
        _CACHE[key] = _build(a_zero)
    return _CACHE[key]


def _prep_inputs(Q, H, a, trans_w, trans_b, q_w, q_b, k_w, k_b, v_w, v_b, W_w):
    """Host-side sharding/layout. Returns (in_maps, a_zero)."""
    a = np.asarray(a, np.float32)
    a_zero = not np.any(a)

    qt_full = np.zeros((D, CP), _BF)
    qt_full[:, :C_FULL] = np.asarray(Q, np.float32).T.astype(_BF)
    ht = np.ascontiguousarray(
        np.asarray(H, np.float32).reshape(BT, D).T.astype(_BF))
    shared = {
        "ht": ht,
        "wtr": np.ascontiguousarray(np.asarray(trans_w, np.float32).T.astype(_BF)),
        "wq": np.ascontiguousarray(np.asarray(q_w, np.float32).T.astype(_BF)),
        "wk": np.ascontiguousarray(np.asarray(k_w, np.float32).T.astype(_BF)),
        "wv": np.ascontiguousarray(np.asarray(v_w, np.float32).T.astype(_BF)),
        "ww": np.ascontiguousarray(np.asarray(W_w, np.float32).T.astype(_BF)),
        "btr": np.asarray(trans_b, np.float32),
        "bq": np.asarray(q_b, np.float32),
        "bk": np.asarray(k_b, np.float32),
        "bvb": np.asarray(v_b, np.float32).reshape(1, TF).astype(_BF),
        "sel": _make_sel(),
    }
    if not a_zero:
        ea = np.exp(a).reshape(B, 4, 128).transpose(2, 0, 1).reshape(128, NTT)
        shared["ea"] = np.ascontiguousarray(ea.astype(np.float32))
    in_maps = []
    for c in range(N_CORES):
        m = dict(shared)
        m["qt"] = np.ascontiguousarray(qt_full[:, c * CS:(c + 1) * CS])
        in_maps.append(m)
    return in_maps, a_zero


def kernel(**inputs) -> np.ndarray:
    from concourse.bass_utils import run_bass_kernel_spmd

    in_maps, a_zero = _prep_inputs(**inputs)
    nc = _get_nc(a_zero)
    res = run_bass_kernel_spmd(nc, in_maps, list(range(N_CORES)))
    out = np.concatenate([res.results[c]["out"] for c in range(N_CORES)], axis=1)
    return np.ascontiguousarray(out[:, :C_FULL])



# revision 2
# speedup vs baseline: 1.3371x; 1.3371x over previous
"""Trainium2 Bass kernel for Co-occurrence Infused Multi-Label Attention, v2.

Shards the n_classes (code) axis across 8 NeuronCores. Key changes vs v1:
  - context contraction runs transposed — out [c-part, z*128+ (64 ctx|den)] —
    so each accumulation step costs 65 N-cols instead of 512, and the
    normalize/z-sum tail is cheap DVE/Pool work instead of selector matmuls
  - software-pipelined emission: scores+exp stream to ACT (the bottleneck
    engine) continuously; K/V/Q transforms, context MMs and tails are
    emitted as fine-grained PE filler between score groups
  - output is [CS, B] (class-major); host transposes

Per core (c = class shard of 1152, z = head, b = chunk, t = token):
  wkt [zh, t]     = tanh(k_wT @ HT + k_b)
  wvp [t, z*65+h] = [tanh(HT.T @ v_wT + v_b); ones]   (ones via one memset)
  per chunk (w in 512,512,128):
    qgt [tf, c]   = tanh(trans_wT @ QT + b_tr)
    qtt [zh, c]   = q_wT @ qgt + q_b
    qwall [c, zh] = qgt.T @ W_wT          (per 128-c block)
    per b, z: psc [t128, 2*w] = wkt_z.T @ qtt_z ; et = Exp(psc)
    per b, cb, zh(4z): yp[c128, z*128+..] += et_slice.T @ wvp_z  (16 MMs)
               prod = yp_ctx * qwall      (Pool)
               num  = reduce_h prod       (DVE)
               recd = 1 / yp_den          (DVE)
               scr[z] = num*recd          (DVE)
    per b, cb: outT_cb[:, b] = reduce_z scr  (DVE)
"""

import numpy as np
import ml_dtypes

C_FULL = 8929
D = 768          # d_model
TF = 512         # transform dim (= NH * DK)
NH = 8           # heads
DK = 64          # head dim
B = 4            # chunks
T = 512          # tokens per chunk
BT = B * T       # 2048
N_CORES = 8
CP = 9216        # padded classes (8 * 1152)
CS = CP // N_CORES   # 1152 classes per core
NDC = D // 128       # 6 d-model chunks
NFC = TF // 128      # 4 transform chunks
NTT = BT // 128      # 16 token tiles
C_CHUNKS = [(0, 512), (512, 512), (1024, 128)]

_BF = ml_dtypes.bfloat16

_CACHE = {}


def _build(a_zero: bool, reps: int = 1, et_bufs: int = 34, pop_rate: int = 1):
    from collections import deque
    from contextlib import ExitStack
    import concourse.bass as bass
    import concourse.mybir as mybir
    import concourse.tile as tile
    from concourse import bacc

    bf = mybir.dt.bfloat16
    f32 = mybir.dt.float32
    AF = mybir.ActivationFunctionType
    ALU = mybir.AluOpType

    nc = bacc.Bacc()

    qt_d = nc.declare_dram_parameter("qt", [D, CS], bf, isOutput=False)
    ht_d = nc.declare_dram_parameter("ht", [D, BT], bf, isOutput=False)
    wtr_d = nc.declare_dram_parameter("wtr", [D, TF], bf, isOutput=False)
    wq_d = nc.declare_dram_parameter("wq", [TF, TF], bf, isOutput=False)
    wk_d = nc.declare_dram_parameter("wk", [D, TF], bf, isOutput=False)
    wv_d = nc.declare_dram_parameter("wv", [D, TF], bf, isOutput=False)
    ww_d = nc.declare_dram_parameter("ww", [TF, TF], bf, isOutput=False)
    btr_d = nc.declare_dram_parameter("btr", [TF], f32, isOutput=False)
    bq_d = nc.declare_dram_parameter("bq", [TF], f32, isOutput=False)
    bk_d = nc.declare_dram_parameter("bk", [TF], f32, isOutput=False)
    bvb_d = nc.declare_dram_parameter("bvb", [1, TF], bf, isOutput=False)
    ab_d = None
    if not a_zero:
        ab_d = nc.declare_dram_parameter("ab", [128, NTT], f32, isOutput=False)
    out_d = nc.declare_dram_parameter("out", [CS, B], f32, isOutput=True)

    with tile.TileContext(nc) as tc, ExitStack() as top:
        const = top.enter_context(tc.tile_pool(name="const", bufs=1))

        w_tr = const.tile([128, NDC * TF], bf)
        w_k = const.tile([128, NDC * TF], bf)
        w_v = const.tile([128, NDC * TF], bf)
        w_q = const.tile([128, NFC * TF], bf)
        w_W = const.tile([128, NFC * TF], bf)
        ht_sb = const.tile([128, NDC * BT], bf)
        b_tr = const.tile([128, NFC], f32)
        b_q = const.tile([128, NFC], f32)
        b_k = const.tile([128, NFC], f32)
        bvb = const.tile([1, TF], bf)
        ones1 = const.tile([1, 128], bf)
        wkt = const.tile([128, NFC * BT], bf)
        wvp = const.tile([128, NTT * 520], bf)

        # chunk-0 qt lives in the const pool so its DMA can lead the sync
        # queue (Qg is the first PE work)
        qt0_sb = const.tile([128, NDC * 512], bf)
        w0 = C_CHUNKS[0][1]
        for jd in range(NDC):
            nc.sync.dma_start(qt0_sb[:, jd * 512: jd * 512 + w0],
                              qt_d[jd * 128:(jd + 1) * 128, 0:w0])
        for j in range(NDC):
            nc.sync.dma_start(w_tr[:, j * TF:(j + 1) * TF], wtr_d[j * 128:(j + 1) * 128, :])
        for j in range(NDC):
            nc.sync.dma_start(ht_sb[:, j * BT:(j + 1) * BT], ht_d[j * 128:(j + 1) * 128, :])
        # scalar queue: biases, w_k (so wkt can start early), then the rest
        nc.scalar.dma_start(b_tr[:], btr_d[:].rearrange("(c p) -> p c", p=128))
        nc.scalar.dma_start(b_k[:], bk_d[:].rearrange("(c p) -> p c", p=128))
        for j in range(NDC):
            nc.scalar.dma_start(w_k[:, j * TF:(j + 1) * TF], wk_d[j * 128:(j + 1) * 128, :])
        for j in range(NFC):
            nc.scalar.dma_start(w_q[:, j * TF:(j + 1) * TF], wq_d[j * 128:(j + 1) * 128, :])
        nc.scalar.dma_start(b_q[:], bq_d[:].rearrange("(c p) -> p c", p=128))
        for j in range(NDC):
            nc.scalar.dma_start(w_v[:, j * TF:(j + 1) * TF], wv_d[j * 128:(j + 1) * 128, :])
        nc.scalar.dma_start(bvb[:], bvb_d[:, :])
        for j in range(NFC):
            nc.scalar.dma_start(w_W[:, j * TF:(j + 1) * TF], ww_d[j * 128:(j + 1) * 128, :])
        nc.gpsimd.memset(ones1[:], 1.0)
        # ones column (h==64 of each z block); tanh writes fill the rest
        nc.gpsimd.memset(wvp[:], 1.0)
        ab_sb = None
        if not a_zero:
            ab_sb = const.tile([128, NTT], f32)
            nc.scalar.dma_start(ab_sb[:], ab_d[:, :])

        with ExitStack() as main:
            qin = main.enter_context(tc.tile_pool(name="qin", bufs=2))
            qg = main.enter_context(tc.tile_pool(name="qg", bufs=2))
            scps = main.enter_context(tc.tile_pool(name="scps", bufs=3, space="PSUM"))
            yps = main.enter_context(tc.tile_pool(name="yps", bufs=2, space="PSUM"))
            etp = main.enter_context(tc.tile_pool(name="etp", bufs=et_bufs))
            prodp = main.enter_context(tc.tile_pool(name="prodp", bufs=3))
            tailp = main.enter_context(tc.tile_pool(name="tailp", bufs=3))
            outp = main.enter_context(tc.tile_pool(name="outp", bufs=2))

            for rep in range(reps):
                fill = deque()

                def pop_fill(k):
                    for _ in range(k):
                        if not fill:
                            return
                        fill.popleft()()

                def flush_fill():
                    while fill:
                        fill.popleft()()

                # ---------- transform units (fine-grained) ----------
                def u_wkt(jz, jp):
                    def f():
                        ps = scps.tile([128, 1024], f32, tag="psc", name="wkps")
                        for half in range(2):
                            jt = jp * 2 + half
                            for jd in range(NDC):
                                nc.tensor.matmul(
                                    ps[:, half * 512: half * 512 + 512],
                                    w_k[:, jd * TF + jz * 128: jd * TF + (jz + 1) * 128],
                                    ht_sb[:, jd * BT + jt * 512: jd * BT + (jt + 1) * 512],
                                    start=(jd == 0), stop=(jd == NDC - 1))
                        nc.scalar.activation(
                            wkt[:, jz * BT + jp * 1024: jz * BT + (jp + 1) * 1024],
                            ps[:, 0:1024], AF.Tanh, bias=b_k[:, jz:jz + 1])
                    return f

                def u_wvp(jt):
                    def f():
                        ps = scps.tile([128, 1024], f32, tag="psc", name="wvps")
                        for jd in range(NDC):
                            nc.tensor.matmul(
                                ps[:, 0:512],
                                ht_sb[:, jd * BT + jt * 128: jd * BT + (jt + 1) * 128],
                                w_v[:, jd * TF:(jd + 1) * TF],
                                start=(jd == 0), stop=False)
                        nc.tensor.matmul(ps[:, 0:512], ones1[0:1, :], bvb[0:1, :],
                                         start=False, stop=True)
                        wvp_z = wvp[:, jt * 520: (jt + 1) * 520].rearrange(
                            "p (z h) -> p z h", h=65)
                        nc.scalar.activation(
                            wvp_z[:, :, 0:64],
                            ps[:, 0:512].rearrange("p (z h) -> p z h", h=64),
                            AF.Tanh)
                    return f

                # ---------- per-chunk phase A units ----------
                def u_qt_dma(st, w):
                    def f():
                        for jd in range(NDC):
                            nc.sync.dma_start(st["qt"][:, jd * 512: jd * 512 + w],
                                              qt_d[jd * 128:(jd + 1) * 128,
                                                   st["c0"]:st["c0"] + w])
                    return f

                def u_qg(st, jf, w):
                    def f():
                        ps = scps.tile([128, 1024], f32, tag="psc", name="qgps")
                        for jd in range(NDC):
                            nc.tensor.matmul(
                                ps[:, :w],
                                w_tr[:, jd * TF + jf * 128: jd * TF + (jf + 1) * 128],
                                st["qt"][:, jd * 512: jd * 512 + w],
                                start=(jd == 0), stop=(jd == NDC - 1))
                        nc.scalar.activation(st["qgt"][:, jf * 512: jf * 512 + w],
                                             ps[:, :w], AF.Tanh, bias=b_tr[:, jf:jf + 1])
                    return f

                def u_qtt(st, jz, w):
                    def f():
                        ps = scps.tile([128, 1024], f32, tag="psc", name="qtps")
                        for jf in range(NFC):
                            nc.tensor.matmul(
                                ps[:, :w],
                                w_q[:, jf * TF + jz * 128: jf * TF + (jz + 1) * 128],
                                st["qgt"][:, jf * 512: jf * 512 + w],
                                start=(jf == 0), stop=(jf == NFC - 1))
                        nc.vector.tensor_scalar_add(st["qtt"][:, jz * 512: jz * 512 + w],
                                                    ps[:, :w], b_q[:, jz:jz + 1])
                    return f

                def u_qwall(st, cb):
                    def f():
                        ps = scps.tile([128, 1024], f32, tag="psc", name="qwps")
                        for jf in range(NFC):
                            nc.tensor.matmul(
                                ps[:, 0:TF],
                                st["qgt"][:, jf * 512 + cb * 128: jf * 512 + (cb + 1) * 128],
                                w_W[:, jf * TF:(jf + 1) * TF],
                                start=(jf == 0), stop=(jf == NFC - 1))
                        nc.vector.tensor_copy(st["qwall"][:, cb * TF:(cb + 1) * TF],
                                              ps[:, 0:TF])
                    return f

                def new_chunk_state(ci):
                    c0, w = C_CHUNKS[ci][0], C_CHUNKS[ci][1]
                    st = {"c0": c0, "w": w, "nb": w // 128, "ci": ci}
                    if ci == 0:
                        st["qt"] = qt0_sb
                    else:
                        st["qt"] = qin.tile([128, NDC * 512], bf, tag="qt", name="qt_sb")
                    st["qgt"] = qg.tile([128, NFC * 512], bf, tag="qgt", name="qgt")
                    st["qtt"] = qg.tile([128, NFC * 512], bf, tag="qtt", name="qtt")
                    st["qwall"] = qg.tile([128, 4 * TF], bf, tag="qwall", name="qwall")
                    st["outts"] = [
                        outp.tile([128, B], f32, tag=f"o{cb}", name=f"outt{cb}")
                        for cb in range(st["nb"])
                    ]
                    return st

                def a_units(st):
                    w = st["w"]
                    us = [u_qt_dma(st, w)]
                    us += [u_qg(st, jf, w) for jf in range(NFC)]
                    us += [u_qtt(st, jz, w) for jz in range(NFC)]
                    us += [u_qwall(st, cb) for cb in range(st["nb"])]
                    return us

                # ---------- attention ----------
                def emit_scores(st, bb):
                    """scores + exp for all z of (chunk, b); returns ets."""
                    w = st["w"]
                    ets = []
                    if w > 128:
                        for z in range(NH):
                            jz, hz = z // 2, (z % 2) * 64
                            pair_et = []
                            for half in range(2):
                                psc = scps.tile([128, 1024], f32, tag="psc", name="psc")
                                for slot in range(2):
                                    jt = half * 2 + slot
                                    nc.tensor.matmul(
                                        psc[:, slot * w: slot * w + w],
                                        wkt[hz:hz + 64,
                                            jz * BT + bb * 512 + jt * 128:
                                            jz * BT + bb * 512 + (jt + 1) * 128],
                                        st["qtt"][hz:hz + 64, jz * 512: jz * 512 + w],
                                        start=True, stop=True)
                                et = etp.tile([128, 1024], bf, tag="et", name="et")
                                if a_zero:
                                    nc.scalar.activation(et[:, 0:2 * w],
                                                         psc[:, 0:2 * w], AF.Exp)
                                else:
                                    for slot in range(2):
                                        jt = half * 2 + slot
                                        gt = bb * 4 + jt
                                        nc.scalar.activation(
                                            et[:, slot * w: slot * w + w],
                                            psc[:, slot * w: slot * w + w],
                                            AF.Exp, bias=ab_sb[:, gt:gt + 1])
                                pair_et.append(et)
                                pop_fill(pop_rate)
                            ets.append(pair_et)
                    else:
                        for zp in range(NH // 2):
                            psc = scps.tile([128, 1024], f32, tag="psc", name="psc")
                            for zi in range(2):
                                z = zp * 2 + zi
                                jz, hz = z // 2, (z % 2) * 64
                                for jt in range(4):
                                    s = zi * 4 + jt
                                    nc.tensor.matmul(
                                        psc[:, s * 128: (s + 1) * 128],
                                        wkt[hz:hz + 64,
                                            jz * BT + bb * 512 + jt * 128:
                                            jz * BT + bb * 512 + (jt + 1) * 128],
                                        st["qtt"][hz:hz + 64, jz * 512: jz * 512 + w],
                                        start=True, stop=True)
                            et = etp.tile([128, 1024], bf, tag="et", name="et")
                            if a_zero:
                                nc.scalar.activation(et[:], psc[:], AF.Exp)
                            else:
                                for zi in range(2):
                                    for jt in range(4):
                                        s = zi * 4 + jt
                                        gt = bb * 4 + jt
                                        nc.scalar.activation(
                                            et[:, s * 128: (s + 1) * 128],
                                            psc[:, s * 128: (s + 1) * 128],
                                            AF.Exp, bias=ab_sb[:, gt:gt + 1])
                            ets.append(et)
                            pop_fill(pop_rate)
                    return ets

                def u_ctx(st, bb, cb, zh, ets, scrs):
                    """context MMs + tail for one (b, c-block, z-half: 4 z)."""
                    w = st["w"]

                    def f():
                        yp = yps.tile([128, 512], f32, tag="y", name="yp")
                        for zi in range(4):
                            z = zh * 4 + zi
                            for jt in range(4):
                                if w > 128:
                                    half, slot = jt // 2, jt % 2
                                    lhsT = ets[z][half][:, slot * w + cb * 128:
                                                        slot * w + (cb + 1) * 128]
                                else:
                                    s = (z % 2) * 4 + jt
                                    lhsT = ets[z // 2][:, s * 128: (s + 1) * 128]
                                gt = bb * 4 + jt
                                nc.tensor.matmul(
                                    yp[:, zi * 128: zi * 128 + 65],
                                    lhsT,
                                    wvp[:, gt * 520 + z * 65: gt * 520 + (z + 1) * 65],
                                    start=(jt == 0), stop=(jt == 3))
                        ypz = yp[:].rearrange("p (z c) -> p z c", c=128)
                        prod = prodp.tile([128, 256], f32, tag="prod", name="prod")
                        nc.vector.tensor_mul(
                            prod[:].rearrange("p (z h) -> p z h", h=64),
                            ypz[:, :, 0:64],
                            st["qwall"][:, cb * TF + zh * 256:
                                        cb * TF + (zh + 1) * 256].rearrange(
                                "p (z h) -> p z h", h=64))
                        num = tailp.tile([128, 4], f32, tag="num", name="num")
                        nc.vector.tensor_reduce(
                            num[:], prod[:].rearrange("p (z h) -> p z h", h=64),
                            mybir.AxisListType.X, ALU.add)
                        recd = tailp.tile([128, 4], f32, tag="recd", name="recd")
                        nc.vector.reciprocal(recd[:], ypz[:, :, 64:65])
                        nc.vector.tensor_mul(scrs[cb][:, zh * 4: (zh + 1) * 4],
                                             num[:], recd[:])
                        if zh == 1:
                            nc.vector.tensor_reduce(
                                st["outts"][cb][:, bb:bb + 1],
                                scrs[cb][:].rearrange("p (o z) -> p o z", o=1),
                                mybir.AxisListType.X, ALU.add)
                            if bb == B - 1:
                                nc.sync.dma_start(
                                    out_d[st["c0"] + cb * 128:
                                          st["c0"] + (cb + 1) * 128, :],
                                    st["outts"][cb][:, :])
                    return f

                def ctx_units(st, bb, ets):
                    scrs = [tailp.tile([128, 8], f32, tag=f"scr{cb}", name="scr")
                            for cb in range(st["nb"])]
                    return [u_ctx(st, bb, cb, zh, ets, scrs)
                            for cb in range(st["nb"]) for zh in range(2)]

                # ---------- prologue ----------
                st0 = new_chunk_state(0)
                if rep > 0:
                    u_qt_dma(st0, st0["w"])()
                for jf in range(NFC):
                    u_qg(st0, jf, st0["w"])()
                u_qtt(st0, 0, st0["w"])()
                u_wkt(0, 0)()
                u_wkt(0, 1)()
                fill.extend([u_wkt(1, 0), u_wkt(1, 1), u_qtt(st0, 1, st0["w"]),
                             u_wkt(2, 0), u_wkt(2, 1), u_qtt(st0, 2, st0["w"]),
                             u_wkt(3, 0), u_wkt(3, 1), u_qtt(st0, 3, st0["w"])])
                fill.extend([u_qwall(st0, cb) for cb in range(st0["nb"])])
                fill.extend([u_wvp(jt) for jt in range(4)])

                # ---------- main pipeline ----------
                st = st0
                nst = None
                for ci in range(len(C_CHUNKS)):
                    for bb in range(B):
                        ets = emit_scores(st, bb)
                        fill.extend(ctx_units(st, bb, ets))
                        if ci == 0 and bb < 3:
                            fill.extend(u_wvp(4 * (bb + 1) + k) for k in range(4))
                        if bb == 1 and ci + 1 < len(C_CHUNKS):
                            nst = new_chunk_state(ci + 1)
                            fill.extend(a_units(nst))
                    flush_fill()
                    st = nst
                    nst = None

    nc.compile()
    return nc


def _get_nc(a_zero: bool):
    key = ("nc", a_zero)
    if key not in _CACHE:
        _CACHE[key] = _build(a_zero)
    return _CACHE[key]


def _prep_inputs(Q, H, a, trans_w, trans_b, q_w, q_b, k_w, k_b, v_w, v_b, W_w):
    """Host-side sharding/layout. Returns (in_maps, a_zero)."""
    a = np.asarray(a, np.float32)
    a_zero = not np.any(a)

    qt_full = np.zeros((D, CP), _BF)
    qt_full[:, :C_FULL] = np.asarray(Q, np.float32).T.astype(_BF)
    ht = np.ascontiguousarray(
        np.asarray(H, np.float32).reshape(BT, D).T.astype(_BF))
    shared = {
        "ht": ht,
        "wtr": np.ascontiguousarray(np.asarray(trans_w, np.float32).T.astype(_BF)),
        "wq": np.ascontiguousarray(np.asarray(q_w, np.float32).T.astype(_BF)),
        "wk": np.ascontiguousarray(np.asarray(k_w, np.float32).T.astype(_BF)),
        "wv": np.ascontiguousarray(np.asarray(v_w, np.float32).T.astype(_BF)),
        "ww": np.ascontiguousarray(np.asarray(W_w, np.float32).T.astype(_BF)),
        "btr": np.asarray(trans_b, np.float32),
        "bq": np.asarray(q_b, np.float32),
        "bk": np.asarray(k_b, np.float32),
        "bvb": np.asarray(v_b, np.float32).reshape(1, TF).astype(_BF),
    }
    if not a_zero:
        ab = a.reshape(B, 4, 128).transpose(2, 0, 1).reshape(128, NTT)
        shared["ab"] = np.ascontiguousarray(ab.astype(np.float32))
    in_maps = []
    for c in range(N_CORES):
        m = dict(shared)
        m["qt"] = np.ascontiguousarray(qt_full[:, c * CS:(c + 1) * CS])
        in_maps.append(m)
    return in_maps, a_zero


def kernel(**inputs) -> np.ndarray:
    from concourse.bass_utils import run_bass_kernel_spmd

    in_maps, a_zero = _prep_inputs(**inputs)
    nc = _get_nc(a_zero)
    res = run_bass_kernel_spmd(nc, in_maps, list(range(N_CORES)))
    out = np.concatenate([res.results[c]["out"] for c in range(N_CORES)], axis=0)
    return np.ascontiguousarray(out.T[:, :C_FULL])
